# revision 44
# baseline (speedup 1.0000x reference)
"""Differential attention (DiffAttn) kernel for 8 TRN2 NeuronCores.

Problem: B=4, T=4096, C=1024, one differential head (2x64 qk dims, 128 v dims),
causal, weights = softmax(q1k1/8) - lam * softmax(q2k2/8), out = weights @ v.

Sharding: pure data-parallel, zero collectives. 8 cores = 4 batches x 2
query-halves. The query rows are zigzag-interleaved at 256-row granularity
(core half h owns rows [512k + 256h, 512k + 256h + 256) for k=0..7) so both
halves have identical causal tile structure (SPMD: one graph for all cores)
and identical FLOPs.

Per-core pipeline (bf16 compute, fp32 accumulation):
  - host pre-swizzles x^T into [128p, 8sb, 8c, 512] with each 512-key block
    ROLLED left by 256*half, so every core's own query columns sit at block
    positions [0, 256) -- the q projection reads them straight out of the
    resident x^T tile (no separate xq upload; key order inside a block is
    irrelevant to sums/PV, and the causal masks are host-built per core for
    the rolled geometry).
  - projections on PE: kT[128f, T], qT[128f, 2048] (feature-major = scores
    operand layout) and v[s, 128] (via vT + DMA-transpose).
  - scores: PE-array packing. Both heads x both query-halves of a subtile
    go through FOUR 64x64 quadrant matmuls (tile_position) that execute
    CONCURRENTLY on the PE -- head1 into PSUM bank set A, head2 into B,
    ~2x the un-packed score throughput.
  - causal mask on the diagonal 512-chunk: PRELOADED into PSUM by packed
    identity matmuls (start=True), score matmuls then accumulate on top
    (start=False; PSUM has_written clearing is per-region -- verified on
    HW). This removes the DVE mask-add from the PE->ACT critical path.
  - score PSUM lives in THREE rotating 2-bank banksets: a group's score
    matmuls write a bankset whose previous exp finished ~3 exp-steps ago,
    so score streams overlap the exps instead of serializing behind the
    bank write-after-read hazard. Exp groups of 2 chunks, accum_out row
    sums (no max-shift needed: scores ~N(0,1)).
  - combine p_neg = p2 * (lam*sum1/sum2) - p1 as a fused DVE op in TWO
    chunk-range pieces; piece 0's xbar DMA-transpose overlaps piece 1's
    DVE pass. PV matmuls enter the PE stream a full subtile after their
    transpose completed (~2us DMA-completion-semaphore latency), PV
    evictions are emitted at controlled points outside the PE filler
    (they'd otherwise head-of-line-block the DVE behind PE progress),
    and the output scale by -1/sum1 is fused into the eviction.
  - per-512-key-block v tiles: a block's transpose-write carries no
    whole-tile WAR hazard against PV reads of other blocks.
  - ALL input loads ride the sync HWDGE ring in priority order: the 16
    DMA engines are shared by all 8 cores, so a parallel flood dilutes
    "block 0 first" 8x; the ring's per-issue serialization makes the
    first k-projection's inputs land ~2us in, chip-wide.
"""
import math
import os
import sys
import types
from contextlib import ExitStack

import ml_dtypes
import numpy as np


def _install_ntff_hook():
    """Make `antenv.axon_hooks` importable (the agent image ships a stub
    antenv without it), wiring the NTFF profile hook straight to the axon
    .so so run_bass_kernel_spmd(trace=True) can report HW exec time."""
    try:
        import antenv.axon_hooks  # noqa: F401
        return
    except Exception:
        pass
    try:
        import antenv
    except Exception:
        return
    mod = types.ModuleType("antenv.axon_hooks")
    mod._hook = None

    def set_axon_ntff_profile_hook(h):
        mod._hook = h

    def get_axon_ntff_profile_hook():
        if mod._hook is None:
            try:
                from trn_agent_boot.trn_boot import _ntff_profile_via_ctypes
                mod._hook = _ntff_profile_via_ctypes("/opt/axon/libaxon_pjrt.so")
            except Exception:
                mod._hook = None
        return mod._hook

    mod.set_axon_ntff_profile_hook = set_axon_ntff_profile_hook
    mod.get_axon_ntff_profile_hook = get_axon_ntff_profile_hook
    sys.modules["antenv.axon_hooks"] = mod
    antenv.axon_hooks = mod


_install_ntff_hook()

import concourse.bacc as bacc
import concourse.bass as bass
import concourse.bass_utils as _bass_utils
import concourse.tile as tile
from concourse import mybir
from concourse.bass_utils import run_bass_kernel_spmd

# zero-egress container: don't try to copy NEFF/NTFF artifacts to a bucket
_bass_utils.upload_artifacts = lambda tmpdir: f"local://{tmpdir}"

BF16 = mybir.dt.bfloat16
F32 = mybir.dt.float32
NPBF16 = ml_dtypes.bfloat16
ts = bass.ts

B, T, C = 4, 4096, 1024
HS, H2 = 64, 128
NSUB = 16          # 128-row query subtiles per core
ROWS = NSUB * 128  # 2048 query rows per core
MASK_NEG = -30000.0
V_MASKMM = True   # causal mask via PSUM-preload identity matmuls
ORDER = list(range(NSUB))

LAST_EXEC_NS = None
_NC_CACHE = {}


def _t0(j, half):
    """Global first query row of subtile j on core-half `half`."""
    return 512 * (j // 2) + 128 * (j % 2) + 256 * half


def _build(lam: float):
    nc = bacc.Bacc()
    xT_e = nc.declare_dram_parameter("xT", [128, 8, 8, 512], BF16, isOutput=False)
    wq_e = nc.declare_dram_parameter("wq", [128, 8, 128], BF16, isOutput=False)
    wk_e = nc.declare_dram_parameter("wk", [128, 8, 128], BF16, isOutput=False)
    wv_e = nc.declare_dram_parameter("wv", [128, 8, 128], BF16, isOutput=False)
    cm_e = nc.declare_dram_parameter("cmask", [128, 2, 512], BF16, isOutput=False)
    id_e = nc.declare_dram_parameter("ident", [128, 128], BF16, isOutput=False)
    out_e = nc.declare_dram_parameter("out", [NSUB, 128, H2], BF16, isOutput=True)

    Exp = mybir.ActivationFunctionType.Exp
    mult = mybir.AluOpType.mult
    sub = mybir.AluOpType.subtract
    add = mybir.AluOpType.add

    with ExitStack() as ctx:
        tc = ctx.enter_context(tile.TileContext(nc))
        const = ctx.enter_context(tc.tile_pool(name="const", bufs=1))
        persist = ctx.enter_context(tc.tile_pool(name="persist", bufs=1))
        vt_pool = ctx.enter_context(tc.tile_pool(name="vt", bufs=2))
        p_pool = ctx.enter_context(tc.tile_pool(name="p", bufs=3))
        pn_pool = ctx.enter_context(tc.tile_pool(name="pn", bufs=3))
        pt_pool = ctx.enter_context(tc.tile_pool(name="pt", bufs=4))
        small = ctx.enter_context(tc.tile_pool(name="small", bufs=4))
        osb_pool = ctx.enter_context(tc.tile_pool(name="osb", bufs=2))
        proj_ps = ctx.enter_context(tc.tile_pool(name="proj_ps", bufs=1, space="PSUM"))
        sc_ps = ctx.enter_context(tc.tile_pool(name="sc_ps", bufs=1, space="PSUM"))
        pv_ps = ctx.enter_context(tc.tile_pool(name="pv_ps", bufs=1, space="PSUM"))

        # --- HAM warm-up on a memset tile: no DMA dependency, PE busy from
        # ~0.5us so the clock gate ramps while the x^T blocks land.
        warm_sb = const.tile([128, 128], BF16)
        nc.gpsimd.memset(warm_sb[:], 0.0)
        warm = sc_ps.tile([128, 2, 512], F32, tag="s0")
        for _ in range(56):
            nc.tensor.matmul(warm[:, 0, 0:128], warm_sb[:], warm_sb[:],
                             start=True, stop=True)

        # --- constants + resident x^T ---
        wq_sb = const.tile([128, 8, 128], BF16)
        wk_sb = const.tile([128, 8, 128], BF16)
        wv_sb = const.tile([128, 8, 128], BF16)
        cm_sb = const.tile([128, 2, 512], BF16)
        id_sb = const.tile([128, 128], BF16)
        # ALL big loads go on the sync HWDGE ring in priority order: the 16
        # DMA engines are SHARED by all 8 cores, so a parallel flood from
        # two rings dilutes "block 0 first" 8x (first k-proj waited ~21us).
        # The ring's ~650ns-per-issue serialization turns the flood into a
        # priority-ordered trickle: xt0/wk/wq complete in ~2us chip-wide.
        xt_sb = const.tile([128, 8, 8, 512], BF16)   # [p, sb, c, col]
        nc.sync.dma_start(xt_sb[:, 0], xT_e[:, 0])
        nc.sync.dma_start(wk_sb[:], wk_e[:])
        nc.sync.dma_start(wq_sb[:], wq_e[:])
        nc.sync.dma_start(xt_sb[:, 1], xT_e[:, 1])
        nc.sync.dma_start(xt_sb[:, 2], xT_e[:, 2])
        nc.sync.dma_start(wv_sb[:], wv_e[:])
        nc.sync.dma_start(cm_sb[:], cm_e[:])
        nc.sync.dma_start(id_sb[:], id_e[:])
        nc.sync.dma_start(xt_sb[:, 3], xT_e[:, 3])
        nc.sync.dma_start(xt_sb[:, 4], xT_e[:, 4])
        nc.sync.dma_start(xt_sb[:, 5], xT_e[:, 5])
        nc.sync.dma_start(xt_sb[:, 6], xT_e[:, 6])
        nc.sync.dma_start(xt_sb[:, 7], xT_e[:, 7])

        # --- persistent projection outputs ---
        qT = persist.tile([128, ROWS], BF16)     # [q-feature, own t]
        kT = persist.tile([128, T], BF16)        # [k-feature, s]
        # one v tile per 512-key block: a block's transpose-write must not
        # pick up write-after-read hazards against PV reads of OTHER blocks
        v_sb = [persist.tile([128, 4, 128], BF16, name=f"v{b}", tag=f"v{b}")
                for b in range(8)]  # [s%128, (s//128)%4, v-feature]

        # PE's per-engine instruction stream is static and in-order, so ALL
        # deferrable PE work (projection matmuls + PV matmuls of earlier
        # subtiles) goes into one FIFO of closures, drained inside the exp
        # windows of the score loop: PE never idles waiting for ACT, and
        # never sees a >3.4us gap (which would drop the HAM clock gate).
        filler = []
        popped = [0]
        appended = [0]

        def push(fn):
            filler.append(fn)
            appended[0] += 1

        def fill(n):
            while n > 0 and filler:
                filler.pop(0)()
                popped[0] += 1
                n -= 1

        def drain_to(mark):
            while popped[0] < mark and filler:
                filler.pop(0)()
                popped[0] += 1

        def proj_block(w_sb, rhs_of_c, done):
            ps_box = []

            def mk(c):
                def go():
                    if c == 0:
                        ps_box.append(proj_ps.tile([128, 512], F32,
                                                   name="pp", tag="pp"))
                    nc.tensor.matmul(ps_box[0][:], w_sb[:, c, :], rhs_of_c(c),
                                     start=(c == 0), stop=(c == 7))
                    if c == 7:
                        done(ps_box[0])
                return go

            for c in range(8):
                push(mk(c))

        def q_done(tb):
            def done(ps):
                nc.vector.tensor_copy(qT[:, ts(tb, 512)], ps[:])
            return done

        def k_done(sb):
            def done(ps):
                nc.vector.tensor_copy(kT[:, ts(sb, 512)], ps[:])
            return done

        def v_done(sb):
            def done(ps):
                vt = vt_pool.tile([128, 512], BF16)
                nc.vector.tensor_copy(vt[:], ps[:])
                nc.sync.dma_start_transpose(v_sb[sb][:], vt[:])
            return done

        # supply schedule; marks record the FIFO position whose drain
        # guarantees kT(sb) / qT(tb) writes are emitted (Tile derives
        # dependencies from emission order, so consumers must be emitted
        # after producers). v-blocks are deferred so the ACT-bound last
        # subtiles still have projection filler for the PE.
        k_mark = {}
        q_mark = {}
        SUPPLY = [("k", 0), ("k", 1), ("q", 0), ("v", 0),
                  ("k", 2), ("q", 1), ("v", 1), ("k", 3), ("v", 2),
                  ("k", 4), ("q", 2), ("v", 3), ("k", 5), ("v", 4),
                  ("k", 6), ("q", 3), ("v", 5), ("k", 7), ("v", 6), ("v", 7)]
        for kind, i in SUPPLY:
            if kind == "k":
                proj_block(wk_sb, lambda c, s=i: xt_sb[:, s, c, :], k_done(i))
                k_mark[i] = appended[0]
            elif kind == "q":
                # q block tb's own-query columns are block positions [0,256)
                # of x^T blocks 2tb and 2tb+1 (host rolls each 512-block by
                # 256*half, making this slice core-independent).
                proj_block(wq_sb,
                           lambda c, t=i: xt_sb[:, 2 * t:2 * t + 2, c, 0:256],
                           q_done(i))
                q_mark[i] = appended[0]
            else:
                proj_block(wv_sb, lambda c, s=i: xt_sb[:, s, c, :], v_done(i))

        # the combine (sums -> gsc -> p_neg) of a subtile is emitted AFTER
        # the next subtile's first exp group: the DVE executes in emission
        # order, so emitting the (up to 4.3us) p_neg pass before the next
        # subtile's first exp would stall that exp behind it every subtile.
        pending_comb = [None]
        pending_pv = []  # (j, pt, r1, nch) awaiting PV queueing
        # score PSUM banksets: 3 rotating pairs of banks. Group g writes a
        # bankset whose previous exp finished ~3 exp-steps ago, so the score
        # matmul streams OVERLAP the current exps instead of serializing
        # behind the bank's write-after-read hazard.
        bs_cnt = [0]

        def next_bank():
            t = sc_ps.tile([128, 2, 512], F32, tag=f"s{bs_cnt[0] % 3}")
            bs_cnt[0] += 1
            return t

        def emit_pending_comb():
            if pending_comb[0] is not None:
                fn = pending_comb[0]
                pending_comb[0] = None
                fn()

        def attention_scores(j):
            nch = j // 2 + 1          # 512-wide key chunks covered
            ngr = (nch + 1) // 2      # 2-chunk exp groups per head
            m = j % 2
            p1 = p_pool.tile([128, nch, 512], BF16, tag="p1")
            p2 = p_pool.tile([128, nch, 512], BF16, tag="p2")
            sp1 = small.tile([128, 4], F32, tag="sp1")
            sp2 = small.tile([128, 4], F32, tag="sp2")
            for gi in range(ngr):
                used = min(2, nch - 2 * gi)
                psA = next_bank()
                psB = next_bank()
                if 2 * gi <= nch - 1 < 2 * gi + used and V_MASKMM:
                    # this group holds the diagonal chunk: preload its causal
                    # mask into PSUM with two FULL-WIDTH identity matmuls
                    # (K=128 uses the whole array, so they strictly serialize
                    # with every other matmul -- no PSUM write races with the
                    # row-tiled score pairs that accumulate on top later).
                    dslot = nch - 1 - 2 * gi
                    nc.tensor.matmul(psA[:, dslot, :], id_sb[:],
                                     cm_sb[:, m, :], start=True, stop=True)
                    nc.tensor.matmul(psB[:, dslot, :], id_sb[:],
                                     cm_sb[:, m, :], start=True, stop=True)
                for qd in range(used):
                    ch = 2 * gi + qd
                    diag = ch == nch - 1
                    st = not (diag and V_MASKMM)
                    # row-tiled score pair: head1 (K rows 0-63) and head2
                    # (K rows 64-127) run CONCURRENTLY on disjoint row
                    # halves of the PE array, each with full M=128 output
                    # into its own PSUM bank.
                    nc.tensor.matmul(psA[:, qd, :],
                                     qT[0:64, ts(j, 128)],
                                     kT[0:64, ts(ch, 512)],
                                     start=st, stop=True,
                                     tile_position=(0, 0))
                    nc.tensor.matmul(psB[:, qd, :],
                                     qT[64:128, ts(j, 128)],
                                     kT[64:128, ts(ch, 512)],
                                     start=st, stop=True,
                                     tile_position=(64, 0))
                for h, (ps, p, sp) in ((0, (psA, p1, sp1)), (1, (psB, p2, sp2))):
                    if 2 * gi + used == nch and not V_MASKMM:
                        nc.vector.tensor_add(ps[:, used - 1, :],
                                             ps[:, used - 1, :],
                                             cm_sb[:, m, :])
                    nc.scalar.activation(p[:, 2 * gi:2 * gi + used, :],
                                         ps[:, 0:used, :], Exp,
                                         accum_out=sp[:, gi:gi + 1])
                    # cover the (used*512+352)/1.2 ns exp window with filler
                    # slots so PE stays busy while ACT runs -- slightly
                    # under-filled: the next score pair must not queue behind
                    # too much filler, or the next exp starts late (ACT is
                    # the finishing engine)
                    fill(max(2, (used * 512 + 352) // 280))
            # previous subtile's combine emits only now, AFTER this
            # subtile's first exps are queued: the DVE is in-order, so the
            # (up to 4.3us) p_neg pass must never precede work that gates
            # an exp on the critical ACT chain
            emit_pending_comb()

            def combine():
                if ngr == 1:
                    sum1, sum2 = sp1[:, 0:1], sp2[:, 0:1]
                else:
                    s1t = small.tile([128, 1], F32, tag="s1t")
                    s2t = small.tile([128, 1], F32, tag="s2t")
                    nc.vector.tensor_reduce(s1t[:], sp1[:, 0:ngr],
                                            axis=mybir.AxisListType.X, op=add)
                    nc.vector.tensor_reduce(s2t[:], sp2[:, 0:ngr],
                                            axis=mybir.AxisListType.X, op=add)
                    sum1, sum2 = s1t[:], s2t[:]
                r2 = small.tile([128, 1], F32, tag="r2")
                r1_t = small.tile([128, 1], F32, tag="r1")
                gsc = small.tile([128, 1], F32, tag="gsc")
                nc.vector.reciprocal(r2[:], sum2)
                nc.vector.reciprocal(r1_t[:], sum1)
                # gsc = lam * sum1 / sum2
                nc.vector.scalar_tensor_tensor(gsc[:], sum1, float(lam),
                                               r2[:], op0=mult, op1=mult)
                # p_neg = p2 * gsc - p1, in two pieces: piece 0's transpose
                # runs on the xbar WHILE the DVE computes piece 1, halving
                # the serial combine->transpose latency for big subtiles.
                ha = (nch + 1) // 2 if nch >= 4 else nch
                pts = []
                for pi, (c0, c1) in enumerate(((0, ha), (ha, nch))):
                    if c0 == c1:
                        continue
                    w = c1 - c0
                    pn = pn_pool.tile([128, 4, 512], BF16, name=f"pn{pi}",
                                      tag=f"pn{pi}")
                    nc.vector.scalar_tensor_tensor(pn[:, 0:w, :],
                                                   p2[:, c0:c1, :], gsc[:],
                                                   p1[:, c0:c1, :],
                                                   op0=mult, op1=sub)
                    pt = pt_pool.tile([128, 16, 128], BF16, name=f"pt{pi}",
                                      tag=f"pt{pi}")
                    nc.sync.dma_start_transpose(pt[:, 0:4 * w, :],
                                                pn[:, 0:w, :])
                    pts.append((pt, 4 * c0))
                pending_pv.append((j, pts, r1_t, nch))

            pending_comb[0] = combine

        # PV output evictions are emitted OUTSIDE the filler FIFO at
        # controlled points: inside the filler they land at arbitrary spots
        # in the DVE's in-order stream and head-of-line-block the next
        # combine behind PE progress (PE->DVE backpressure -> ACT stall).
        pending_evicts = []  # (last_push_index, emit_fn)

        def emit_ready_evicts():
            while pending_evicts and pending_evicts[0][0] <= popped[0]:
                pending_evicts.pop(0)[1]()

        def queue_pv(j, pts, r1, nch):
            # the single pv bank: the previous chain's eviction must be
            # emitted before this chain's start=True matmul (WAR). Force
            # its matmuls out of the FIFO if pops haven't reached them.
            while pending_evicts:
                last_push, fn = pending_evicts.pop(0)
                drain_to(last_push)
                fn()
            pv_box = []

            def pt_of(cc):
                for pt, cc0 in reversed(pts):
                    if cc >= cc0:
                        return pt, cc - cc0
                raise AssertionError

            def mk_mm(cc):
                def go():
                    if cc == 0:
                        pv_box.append(pv_ps.tile([128, 128], F32,
                                                 name="pv", tag="pv"))
                    pt, off = pt_of(cc)
                    nc.tensor.matmul(pv_box[0][:], pt[:, off, :],
                                     v_sb[cc // 4][:, cc % 4, :],
                                     start=(cc == 0), stop=(cc == 4 * nch - 1))
                return go

            def finish():
                osb = osb_pool.tile([128, 128], BF16)
                # out = pv * r1 * (-1)  (fused negate undoes the p_neg sign)
                nc.vector.tensor_scalar(osb[:], pv_box[0][:], r1[:], -1.0,
                                        op0=mult, op1=mult)
                nc.gpsimd.dma_start(out_e[j, :, :], osb[:])

            for cc in range(4 * nch):
                push(mk_mm(cc))
            pending_evicts.append((appended[0], finish))

        def run_subtile(j):
            # kT/qT producer closures for this subtile must be emitted first
            drain_to(max(k_mark[j // 2], q_mark[j // 4]))
            emit_ready_evicts()
            # keep one chain pending: PV(x) enters the PE stream a full
            # subtile after pt(x)'s transpose completed (~2.2us completion
            # latency on the DMA semaphore), so PE never head-of-line
            # stalls on it mid-score-stream.
            while len(pending_pv) >= 2:
                queue_pv(*pending_pv.pop(0))
            attention_scores(j)
            fill(4)

        for j in ORDER:
            run_subtile(j)
        emit_pending_comb()
        # tail drain: dummy warm matmuls before each PV chain keep the PE
        # busy through the ~2us transpose-completion-semaphore waits, so
        # the HAM clock gate stays at full rate for the final PV chains
        wfin = sc_ps.tile([128, 2, 512], F32, tag="s0")
        while pending_pv:
            for _ in range(16):
                nc.tensor.matmul(wfin[:, 0, 0:128], warm_sb[:], warm_sb[:],
                                 start=True, stop=True)
            queue_pv(*pending_pv.pop(0))
            fill(len(filler))
            emit_ready_evicts()
        fill(len(filler))
        emit_ready_evicts()

    nc.compile()
    return nc


def _lambda_init(depth):
    return 0.8 - 0.6 * math.exp(-0.3 * (depth + 1))


def kernel(x, Wq, Wk, Wv, lambda_q1, lambda_q2, lambda_k1, lambda_k2):
    global LAST_EXEC_NS
    x = np.asarray(x, dtype=np.float32)
    Wq = np.asarray(Wq, dtype=np.float32)
    Wk = np.asarray(Wk, dtype=np.float32)
    Wv = np.asarray(Wv, dtype=np.float32)
    lq1 = np.asarray(lambda_q1, dtype=np.float64)
    lq2 = np.asarray(lambda_q2, dtype=np.float64)
    lk1 = np.asarray(lambda_k1, dtype=np.float64)
    lk2 = np.asarray(lambda_k2, dtype=np.float64)

    lam = float(np.exp(np.dot(lq1, lk1)) - np.exp(np.dot(lq2, lk2))
                + _lambda_init(0))

    key = round(lam, 9)
    if key not in _NC_CACHE:
        _NC_CACHE[key] = _build(lam)
    nc = _NC_CACHE[key]

    def _wswz(w):  # [C, H2] -> [128p, 8c, 128]
        return np.ascontiguousarray(
            w.astype(NPBF16).reshape(8, 128, 128).transpose(1, 0, 2))

    wq_h = _wswz(Wq * 0.125)
    wk_h = _wswz(Wk)
    wv_h = _wswz(Wv)

    ident = np.zeros((128, 128), dtype=NPBF16)
    ident[np.arange(128), np.arange(128)] = 1.0

    # x^T per (batch, half) in bf16: each 512-key block rolled left by
    # 256*half, then swizzled to [128p, 8sb, 8c, 512]
    xT = [x[b].T.astype(NPBF16) for b in range(B)]

    def _xt_swz(t):
        return np.ascontiguousarray(
            t.reshape(8, 128, 8, 512).transpose(1, 2, 0, 3))

    # causal masks for the rolled geometry: query row r of an m-subtile is
    # global t = 512k + 256h + 128m + r; diag-chunk position c holds key
    # 512k + ((c + 256h) mod 512); valid iff that key <= t.
    p_idx = np.arange(128)[:, None]
    c_idx = np.arange(512)[None, :]
    in_maps = []
    for core in range(8):
        b, half = core // 2, core % 2
        t = xT[b]
        if half == 1:
            t = t.reshape(C, 8, 2, 256)[:, :, ::-1, :].reshape(C, T)
        cm = np.empty((128, 2, 512), dtype=NPBF16)
        keypos = (c_idx + 256 * half) % 512
        for m in range(2):
            valid = keypos <= 256 * half + 128 * m + p_idx
            cm[:, m, :] = np.where(valid, 0.0, MASK_NEG).astype(NPBF16)
        in_maps.append({"xT": _xt_swz(t), "wq": wq_h, "wk": wk_h,
                        "wv": wv_h, "cmask": cm, "ident": ident})

    try:
        res = run_bass_kernel_spmd(nc, in_maps, list(range(8)))
    except Exception:
        # transient device errors (and occasional profiling-path failures)
        # succeed on retry; drop tracing for the rerun if it was on
        if os.environ.get("BASS_TRACE"):
            os.environ["BASS_NEVER_TRACE"] = "1"
        res = run_bass_kernel_spmd(nc, in_maps, list(range(8)))
    LAST_EXEC_NS = res.exec_time_ns

    out = np.empty((B, T, H2), dtype=np.float32)
    for core in range(8):
        b, half = core // 2, core % 2
        o = np.asarray(res.results[core]["out"]).astype(np.float32)
        for j in range(NSUB):
            t0 = _t0(j, half)
            out[b, t0:t0 + 128, :] = o[j]
    return out
